# revision 2
# baseline (speedup 1.0000x reference)
"""BitLinear-1.58 forward on 8 trn2 NeuronCores.

out = x @ qw.T + bias, qw = clip(round(w / (eps + mean|w|)), -1, 1).

Strategy (fp8 DoubleRow, 2x PE rate vs bf16):
  - Quantize the weight on host (bit-identical to the reference), transpose
    to [in, out]; ternary values are exact in fp8 e4m3.
  - Quantize x to e4m3. Plain e4m3 x gives rel_err ~2.3e-2 (> the 2e-2
    gate), so augment the contraction: for the first C_CORR of the 2048
    K-indices, append residual rows 64*(x - e4m3(x)) paired with weight rows
    w/64 (both exact in e4m3). This cuts the error to ~1.65e-2 at
    (2048+C_CORR)/2048 of the fp8 cost.
  - Column-parallel across 8 cores: each core computes the full-token output
    for a 1024-wide slice of out_features with a Bass/Tile PE matmul
    (fp8 e4m3 inputs, DoubleRow perf mode, fp32 PSUM accumulation).
  - Concatenate the 8 output slices.
"""

import numpy as np
import ml_dtypes

B, S, IN, OUT = 4, 2048, 2048, 8192
N_CORES = 8
TOK = B * S
N_SHARD = OUT // N_CORES
SCALE_EPS = 1e-05

C_CORR = 1024  # K-indices that get an exact-residual correction row
K_AUG = IN + C_CORR
RESID_SCALE = 64.0  # power of two: w/RESID_SCALE stays exact in e4m3

_CACHED_NC = None


def _build_nc():
    import concourse.mybir as mybir
    import concourse.tile as tile
    from concourse import bacc
    from concourse.kernels.tile_matmul import matmul_tile_kernel

    nc = bacc.Bacc(None, target_bir_lowering=False)

    x_t = nc.dram_tensor("x_t", [K_AUG, TOK], mybir.dt.float8e4, kind="ExternalInput")
    w_t = nc.dram_tensor(
        "w_t", [K_AUG, N_SHARD], mybir.dt.float8e4, kind="ExternalInput"
    )
    out = nc.dram_tensor("out", [TOK, N_SHARD], mybir.dt.float32, kind="ExternalOutput")

    with tile.TileContext(nc) as tc:
        # PE warm-up: dummy matmuls with no data deps run while the first
        # input tiles are still DMA-ing in, so the HAM clock gate is already
        # released (2.4 GHz) when the real matmul stream starts.
        with (
            tc.tile_pool(name="warm", bufs=1) as warm_pool,
            tc.tile_pool(name="warm_psum", bufs=1, space="PSUM") as warm_psum,
        ):
            wl = warm_pool.tile([128, 512], mybir.dt.bfloat16)
            wp = warm_psum.tile([128, 512], mybir.dt.float32)
            nc.vector.memset(wl[:], 0.0)
            n_warm = 14
            for i in range(n_warm):
                nc.tensor.matmul(
                    wp[:], wl[:, :128], wl[:], start=(i == 0), stop=(i == n_warm - 1)
                )

        matmul_tile_kernel(
            tc,
            x_t[:, :],
            w_t[:, :],
            out[:, :],
            MAX_K_TILE_SIZE=256,
        )

    nc.compile()
    return nc


def _get_nc():
    global _CACHED_NC
    if _CACHED_NC is None:
        _CACHED_NC = _build_nc()
    return _CACHED_NC


def _quantize_weight(weight: np.ndarray) -> np.ndarray:
    """Ternarize exactly as the reference does (same jax ops, same backend)."""
    import jax.numpy as jnp

    w = jnp.asarray(weight)
    scale = SCALE_EPS + jnp.mean(jnp.abs(w))
    quant = jnp.clip(jnp.round(w / scale), -1.0, 1.0)
    return np.asarray(quant, dtype=np.float32)


def _prepare_in_maps(x: np.ndarray, weight: np.ndarray):
    f8 = ml_dtypes.float8_e4m3
    qw = _quantize_weight(weight)  # [OUT, IN] ternary fp32

    # weight side: [IN, OUT] ternary plus [C_CORR, OUT] scaled-down copy
    w_t = np.ascontiguousarray(qw.T)  # [IN, OUT] fp32
    w_aug = np.empty((K_AUG, OUT), dtype=f8)
    w_aug[:IN] = w_t.astype(f8)  # exact: -1/0/1
    w_aug[IN:] = (w_t[:C_CORR] / RESID_SCALE).astype(f8)  # exact: +-2^-6

    # x side: [IN, TOK] e4m3 plus [C_CORR, TOK] scaled residual
    x_t = np.ascontiguousarray(x.reshape(TOK, IN).T)  # [IN, TOK] fp32
    x8 = x_t.astype(f8)
    x_aug = np.empty((K_AUG, TOK), dtype=f8)
    x_aug[:IN] = x8
    resid = (x_t[:C_CORR] - x8[:C_CORR].astype(np.float32)) * RESID_SCALE
    x_aug[IN:] = resid.astype(f8)

    return [
        {
            "x_t": x_aug,
            "w_t": np.ascontiguousarray(w_aug[:, i * N_SHARD : (i + 1) * N_SHARD]),
        }
        for i in range(N_CORES)
    ]


def _postprocess(outs: list, bias: np.ndarray) -> np.ndarray:
    out = np.concatenate([np.asarray(o) for o in outs], axis=1)  # [TOK, OUT] f32
    out = out.reshape(B, S, OUT)
    if np.any(bias):
        out = out + bias.astype(np.float32)
    return out


def _ensure_ntff_hook_shim():
    """concourse's trace path imports antenv.axon_hooks, which is missing in
    this image. Provide the same ctypes-based hook (see trn_agent_boot) so a
    globally-set BASS_TRACE can't crash the run."""
    import sys

    try:
        import antenv.axon_hooks  # noqa: F401
        return
    except ImportError:
        pass

    import contextlib
    import ctypes
    import types

    def _make_hook():
        try:
            lib = ctypes.CDLL("/opt/axon/libaxon_pjrt.so")
        except OSError:
            return None
        if not hasattr(lib, "axon_start_nrt_profile"):
            return None
        lib.axon_start_nrt_profile.argtypes = [
            ctypes.POINTER(ctypes.c_int64), ctypes.c_size_t,
        ]
        lib.axon_start_nrt_profile.restype = ctypes.c_int64
        lib.axon_stop_nrt_profile.argtypes = [ctypes.c_char_p]
        lib.axon_stop_nrt_profile.restype = ctypes.c_int64

        @contextlib.contextmanager
        def _hook(output_dir, device_ids):
            import jax

            jax.devices()
            if device_ids:
                ids = (ctypes.c_int64 * len(device_ids))(*device_ids)
                rc = lib.axon_start_nrt_profile(ids, len(device_ids))
            else:
                rc = lib.axon_start_nrt_profile(None, 0)
            if rc != 0:
                raise RuntimeError(f"axon_start_nrt_profile rc={rc}")
            try:
                yield
            finally:
                lib.axon_stop_nrt_profile(str(output_dir).encode())

        return _hook

    hook = _make_hook()
    mod = types.ModuleType("antenv.axon_hooks")
    mod.get_axon_ntff_profile_hook = lambda: hook
    mod.set_axon_ntff_profile_hook = lambda h: None
    sys.modules["antenv.axon_hooks"] = mod
    try:
        import antenv

        antenv.axon_hooks = mod
    except ImportError:
        pass


def kernel(x: np.ndarray, weight: np.ndarray, bias: np.ndarray) -> np.ndarray:
    from concourse.bass_utils import run_bass_kernel_spmd

    x = np.asarray(x, dtype=np.float32)
    weight = np.asarray(weight, dtype=np.float32)
    bias = np.asarray(bias, dtype=np.float32)

    _ensure_ntff_hook_shim()
    in_maps = _prepare_in_maps(x, weight)
    nc = _get_nc()
    try:
        res = run_bass_kernel_spmd(nc, in_maps, core_ids=list(range(N_CORES)))
    except Exception:
        # transient NRT execute failures have been observed to clear on retry
        import time as _time

        _time.sleep(5)
        res = run_bass_kernel_spmd(nc, in_maps, core_ids=list(range(N_CORES)))
    return _postprocess([r["out"] for r in res.results], bias)


# revision 3
# speedup vs baseline: 1.1383x; 1.1383x over previous
"""BitLinear-1.58 forward on 8 trn2 NeuronCores.

out = x @ qw.T + bias, qw = clip(round(w / (eps + mean|w|)), -1, 1).

Strategy (fp8 DoubleRow, 2x PE rate vs bf16):
  - Quantize the weight on host (bit-identical to the reference), transpose
    to [in, out]; ternary values are exact in fp8 e4m3.
  - Quantize x to e4m3. Plain e4m3 x gives rel_err ~2.3e-2 (> the 2e-2
    gate), so augment the contraction: for the first C_CORR of the 2048
    K-indices, append residual rows 64*(x - e4m3(x)) paired with weight rows
    w/64 (both exact in e4m3). This cuts the error to ~1.65e-2 at
    (2048+C_CORR)/2048 of the fp8 cost.
  - Column-parallel across 8 cores: each core computes the full-token output
    for a 1024-wide slice of out_features with a Bass/Tile PE matmul
    (fp8 e4m3 inputs, DoubleRow perf mode, fp32 PSUM accumulation).
  - Concatenate the 8 output slices.
"""

import numpy as np
import ml_dtypes

B, S, IN, OUT = 4, 2048, 2048, 8192
N_CORES = 8
TOK = B * S
N_SHARD = OUT // N_CORES
SCALE_EPS = 1e-05

C_CORR = 1024  # K-indices that get an exact-residual correction row
K_AUG = IN + C_CORR
RESID_SCALE = 64.0  # power of two: w/RESID_SCALE stays exact in e4m3

_CACHED_NC = None


def _build_nc():
    import concourse.mybir as mybir
    import concourse.tile as tile
    from concourse import bacc

    KO = K_AUG // 128  # 128-partition K slabs
    NS = KO // 2  # DoubleRow K slabs (256 logical K each)
    MT = TOK // 512  # token tiles
    TS = 4  # 128-token subtiles per token tile
    NH = N_SHARD // 512  # 512-wide output column halves

    nc = bacc.Bacc(None, target_bir_lowering=False)

    x_t = nc.dram_tensor("x_t", [K_AUG, TOK], mybir.dt.float8e4, kind="ExternalInput")
    w_t = nc.dram_tensor(
        "w_t", [K_AUG, N_SHARD], mybir.dt.float8e4, kind="ExternalInput"
    )
    out = nc.dram_tensor("out", [TOK, N_SHARD], mybir.dt.float32, kind="ExternalOutput")

    xv = x_t[:, :].rearrange("(po pi) t -> pi po t", pi=128)  # [128, KO, TOK]
    wv = w_t[:, :].rearrange("(po pi) n -> pi po n", pi=128)  # [128, KO, N_SHARD]
    ov = out[:, :].rearrange(
        "(mo ts p) n -> p mo ts n", ts=TS, p=128
    )  # [128, MT, TS, N_SHARD]

    DR = mybir.MatmulPerfMode.DoubleRow

    with tile.TileContext(nc) as tc:
        with (
            tc.tile_pool(name="warm", bufs=1) as warm_pool,
            tc.tile_pool(name="wpool", bufs=1) as wpool,
            tc.tile_pool(name="xpool", bufs=3) as xpool,
            tc.tile_pool(name="opool", bufs=3) as opool,
            tc.tile_pool(name="psum", bufs=2, space="PSUM") as psum,
        ):
            # The full weight shard stays SBUF-resident (KO KB per partition).
            w_sb = wpool.tile([128, KO, N_SHARD], mybir.dt.float8e4)
            nc.sync.dma_start(w_sb[:], wv)

            # PE warm-up: dummy matmuls with no data deps run while the first
            # input tiles are still DMA-ing in, so the HAM clock gate is
            # already released (2.4 GHz) when the real matmul stream starts.
            # The warm-up psum tile comes from the same rotating pool the
            # real accumulations use, keeping total PSUM usage at 8 banks.
            wl = warm_pool.tile([128, 512], mybir.dt.bfloat16)
            wp = psum.tile([128, 512], mybir.dt.float32, name="ps0")
            nc.vector.memset(wl[:], 0.0)
            n_warm = 18
            for i in range(n_warm):
                nc.tensor.matmul(
                    wp[:], wl[:, :128], wl[:], start=(i == 0), stop=(i == n_warm - 1)
                )

            for mt in range(MT):
                xt = xpool.tile([128, KO, 512], mybir.dt.float8e4, name="xt")
                nc.sync.dma_start(xt[:], xv[:, :, mt * 512 : (mt + 1) * 512])
                for nh in range(NH):
                    pss = [
                        psum.tile([128, 512], mybir.dt.float32, name=f"ps{t}")
                        for t in range(TS)
                    ]
                    for ks in range(NS):
                        for t in range(TS):
                            nc.tensor.matmul(
                                pss[t],
                                xt[:, 2 * ks : 2 * ks + 2, t * 128 : (t + 1) * 128],
                                w_sb[:, 2 * ks : 2 * ks + 2, nh * 512 : (nh + 1) * 512],
                                start=(ks == 0),
                                stop=(ks == NS - 1),
                                perf_mode=DR,
                            )
                    ot = opool.tile([128, TS, 512], mybir.dt.float32, name="ot")
                    for t in range(TS):
                        if t % 2 == 0:
                            nc.scalar.copy(out=ot[:, t, :], in_=pss[t][:])
                        else:
                            nc.vector.tensor_copy(out=ot[:, t, :], in_=pss[t][:])
                    nc.sync.dma_start(
                        ov[:, mt, :, nh * 512 : (nh + 1) * 512], ot[:]
                    )

    nc.compile()
    return nc


def _get_nc():
    global _CACHED_NC
    if _CACHED_NC is None:
        _CACHED_NC = _build_nc()
    return _CACHED_NC


def _quantize_weight(weight: np.ndarray) -> np.ndarray:
    """Ternarize exactly as the reference does (same jax ops, same backend)."""
    import jax.numpy as jnp

    w = jnp.asarray(weight)
    scale = SCALE_EPS + jnp.mean(jnp.abs(w))
    quant = jnp.clip(jnp.round(w / scale), -1.0, 1.0)
    return np.asarray(quant, dtype=np.float32)


def _prepare_in_maps(x: np.ndarray, weight: np.ndarray):
    f8 = ml_dtypes.float8_e4m3
    qw = _quantize_weight(weight)  # [OUT, IN] ternary fp32

    # weight side: [IN, OUT] ternary plus [C_CORR, OUT] scaled-down copy
    w_t = np.ascontiguousarray(qw.T)  # [IN, OUT] fp32
    w_aug = np.empty((K_AUG, OUT), dtype=f8)
    w_aug[:IN] = w_t.astype(f8)  # exact: -1/0/1
    w_aug[IN:] = (w_t[:C_CORR] / RESID_SCALE).astype(f8)  # exact: +-2^-6

    # x side: [IN, TOK] e4m3 plus [C_CORR, TOK] scaled residual
    x_t = np.ascontiguousarray(x.reshape(TOK, IN).T)  # [IN, TOK] fp32
    x8 = x_t.astype(f8)
    x_aug = np.empty((K_AUG, TOK), dtype=f8)
    x_aug[:IN] = x8
    resid = (x_t[:C_CORR] - x8[:C_CORR].astype(np.float32)) * RESID_SCALE
    x_aug[IN:] = resid.astype(f8)

    return [
        {
            "x_t": x_aug,
            "w_t": np.ascontiguousarray(w_aug[:, i * N_SHARD : (i + 1) * N_SHARD]),
        }
        for i in range(N_CORES)
    ]


def _postprocess(outs: list, bias: np.ndarray) -> np.ndarray:
    out = np.concatenate([np.asarray(o) for o in outs], axis=1)  # [TOK, OUT] f32
    out = out.reshape(B, S, OUT)
    if np.any(bias):
        out = out + bias.astype(np.float32)
    return out


def _ensure_ntff_hook_shim():
    """concourse's trace path imports antenv.axon_hooks, which is missing in
    this image. Provide the same ctypes-based hook (see trn_agent_boot) so a
    globally-set BASS_TRACE can't crash the run."""
    import sys

    try:
        import antenv.axon_hooks  # noqa: F401
        return
    except ImportError:
        pass

    import contextlib
    import ctypes
    import types

    def _make_hook():
        try:
            lib = ctypes.CDLL("/opt/axon/libaxon_pjrt.so")
        except OSError:
            return None
        if not hasattr(lib, "axon_start_nrt_profile"):
            return None
        lib.axon_start_nrt_profile.argtypes = [
            ctypes.POINTER(ctypes.c_int64), ctypes.c_size_t,
        ]
        lib.axon_start_nrt_profile.restype = ctypes.c_int64
        lib.axon_stop_nrt_profile.argtypes = [ctypes.c_char_p]
        lib.axon_stop_nrt_profile.restype = ctypes.c_int64

        @contextlib.contextmanager
        def _hook(output_dir, device_ids):
            import jax

            jax.devices()
            if device_ids:
                ids = (ctypes.c_int64 * len(device_ids))(*device_ids)
                rc = lib.axon_start_nrt_profile(ids, len(device_ids))
            else:
                rc = lib.axon_start_nrt_profile(None, 0)
            if rc != 0:
                raise RuntimeError(f"axon_start_nrt_profile rc={rc}")
            try:
                yield
            finally:
                lib.axon_stop_nrt_profile(str(output_dir).encode())

        return _hook

    hook = _make_hook()
    mod = types.ModuleType("antenv.axon_hooks")
    mod.get_axon_ntff_profile_hook = lambda: hook
    mod.set_axon_ntff_profile_hook = lambda h: None
    sys.modules["antenv.axon_hooks"] = mod
    try:
        import antenv

        antenv.axon_hooks = mod
    except ImportError:
        pass


def kernel(x: np.ndarray, weight: np.ndarray, bias: np.ndarray) -> np.ndarray:
    from concourse.bass_utils import run_bass_kernel_spmd

    x = np.asarray(x, dtype=np.float32)
    weight = np.asarray(weight, dtype=np.float32)
    bias = np.asarray(bias, dtype=np.float32)

    _ensure_ntff_hook_shim()
    in_maps = _prepare_in_maps(x, weight)
    nc = _get_nc()
    try:
        res = run_bass_kernel_spmd(nc, in_maps, core_ids=list(range(N_CORES)))
    except Exception:
        # transient NRT execute failures have been observed to clear on retry
        import time as _time

        _time.sleep(5)
        res = run_bass_kernel_spmd(nc, in_maps, core_ids=list(range(N_CORES)))
    return _postprocess([r["out"] for r in res.results], bias)


# revision 6
# speedup vs baseline: 1.2218x; 1.0734x over previous
"""BitLinear-1.58 forward on 8 trn2 NeuronCores.

out = x @ qw.T + bias, qw = clip(round(w / (eps + mean|w|)), -1, 1).

Strategy (fp8 DoubleRow, 2x PE rate vs bf16):
  - Quantize the weight on host (bit-identical to the reference), transpose
    to [in, out]; ternary values are exact in fp8 e4m3.
  - Quantize x to e4m3. Plain e4m3 x gives rel_err ~2.3e-2 (> the 2e-2
    gate), so augment the contraction: for the first C_CORR of the 2048
    K-indices, append residual rows 64*(x - e4m3(x)) paired with weight rows
    w/64 (both exact in e4m3). This cuts the error to ~1.65e-2 at
    (2048+C_CORR)/2048 of the fp8 cost.
  - Column-parallel across 8 cores: each core computes the full-token output
    for a 1024-wide slice of out_features with a Bass/Tile PE matmul
    (fp8 e4m3 inputs, DoubleRow perf mode, fp32 PSUM accumulation).
  - Concatenate the 8 output slices.
"""

import numpy as np
import ml_dtypes

B, S, IN, OUT = 4, 2048, 2048, 8192
N_CORES = 8
TOK = B * S
N_SHARD = OUT // N_CORES
SCALE_EPS = 1e-05

C_CORR = 768  # K-indices that get an exact-residual correction row
K_AUG = IN + C_CORR
RESID_SCALE = 64.0  # power of two: w/RESID_SCALE stays exact in e4m3

_CACHED_NC = None


def _build_nc():
    import concourse.mybir as mybir
    import concourse.tile as tile
    from concourse import bacc

    KO = K_AUG // 128  # 128-partition K slabs
    NS = KO // 2  # DoubleRow K slabs (256 logical K each)
    MT = TOK // 512  # token tiles
    TS = 4  # 128-token subtiles per token tile
    NH = N_SHARD // 512  # 512-wide output column halves

    nc = bacc.Bacc(None, target_bir_lowering=False)

    x_t = nc.dram_tensor("x_t", [K_AUG, TOK], mybir.dt.float8e4, kind="ExternalInput")
    w_t = nc.dram_tensor(
        "w_t", [K_AUG, N_SHARD], mybir.dt.float8e4, kind="ExternalInput"
    )
    out = nc.dram_tensor("out", [TOK, N_SHARD], mybir.dt.float32, kind="ExternalOutput")

    xv = x_t[:, :].rearrange("(po pi) t -> pi po t", pi=128)  # [128, KO, TOK]
    wv = w_t[:, :].rearrange("(po pi) n -> pi po n", pi=128)  # [128, KO, N_SHARD]
    ov = out[:, :].rearrange(
        "(mo ts p) n -> p mo ts n", ts=TS, p=128
    )  # [128, MT, TS, N_SHARD]

    DR = mybir.MatmulPerfMode.DoubleRow

    with tile.TileContext(nc) as tc:
        with (
            tc.tile_pool(name="warm", bufs=1) as warm_pool,
            tc.tile_pool(name="wpool", bufs=1) as wpool,
            tc.tile_pool(name="xpool", bufs=3) as xpool,
            tc.tile_pool(name="opool", bufs=3) as opool,
            tc.tile_pool(name="psum", bufs=2, space="PSUM") as psum,
        ):
            # The full weight shard stays SBUF-resident (KO KB per partition).
            # DMA it in column halves so the first matmul round only waits
            # for the half it consumes (plus the first x tile).
            w_sb = wpool.tile([128, KO, N_SHARD], mybir.dt.float8e4)
            for nh in range(NH):
                nc.sync.dma_start(
                    w_sb[:, :, nh * 512 : (nh + 1) * 512],
                    wv[:, :, nh * 512 : (nh + 1) * 512],
                )

            # PE warm-up: dummy matmuls with no data deps run while the first
            # input tiles are still DMA-ing in, so the HAM clock gate is
            # already released (2.4 GHz) when the real matmul stream starts.
            # The warm-up psum tile comes from the same rotating pool the
            # real accumulations use, keeping total PSUM usage at 8 banks.
            wl = warm_pool.tile([128, 512], mybir.dt.bfloat16)
            wp = psum.tile([128, 512], mybir.dt.float32, name="ps0")
            nc.vector.memset(wl[:], 0.0)
            n_warm = 24
            for i in range(n_warm):
                nc.tensor.matmul(
                    wp[:], wl[:, :128], wl[:], start=(i == 0), stop=(i == n_warm - 1)
                )

            for mt in range(MT):
                xt = xpool.tile([128, KO, 512], mybir.dt.float8e4, name="xt")
                nc.sync.dma_start(xt[:], xv[:, :, mt * 512 : (mt + 1) * 512])
                for nh in range(NH):
                    pss = [
                        psum.tile([128, 512], mybir.dt.float32, name=f"ps{t}")
                        for t in range(TS)
                    ]
                    for ks in range(NS):
                        for t in range(TS):
                            nc.tensor.matmul(
                                pss[t],
                                xt[:, 2 * ks : 2 * ks + 2, t * 128 : (t + 1) * 128],
                                w_sb[:, 2 * ks : 2 * ks + 2, nh * 512 : (nh + 1) * 512],
                                start=(ks == 0),
                                stop=(ks == NS - 1),
                                perf_mode=DR,
                            )
                    ot = opool.tile([128, TS, 512], mybir.dt.float32, name="ot")
                    for t in range(TS):
                        if t % 2 == 0:
                            nc.scalar.copy(out=ot[:, t, :], in_=pss[t][:])
                        else:
                            nc.vector.tensor_copy(out=ot[:, t, :], in_=pss[t][:])
                    nc.sync.dma_start(
                        ov[:, mt, :, nh * 512 : (nh + 1) * 512], ot[:]
                    )

    nc.compile()
    return nc


def _get_nc():
    global _CACHED_NC
    if _CACHED_NC is None:
        _CACHED_NC = _build_nc()
    return _CACHED_NC


def _quantize_weight(weight: np.ndarray) -> np.ndarray:
    """Ternarize exactly as the reference does (same jax ops, same backend)."""
    import jax.numpy as jnp

    w = jnp.asarray(weight)
    scale = SCALE_EPS + jnp.mean(jnp.abs(w))
    quant = jnp.clip(jnp.round(w / scale), -1.0, 1.0)
    return np.asarray(quant, dtype=np.float32)


def _prepare_in_maps(x: np.ndarray, weight: np.ndarray):
    f8 = ml_dtypes.float8_e4m3
    qw = _quantize_weight(weight)  # [OUT, IN] ternary fp32

    # weight side: [IN, OUT] ternary plus [C_CORR, OUT] scaled-down copy
    w_t = np.ascontiguousarray(qw.T)  # [IN, OUT] fp32
    w_aug = np.empty((K_AUG, OUT), dtype=f8)
    w_aug[:IN] = w_t.astype(f8)  # exact: -1/0/1
    w_aug[IN:] = (w_t[:C_CORR] / RESID_SCALE).astype(f8)  # exact: +-2^-6

    # x side: [IN, TOK] e4m3 plus [C_CORR, TOK] scaled residual
    x_t = np.ascontiguousarray(x.reshape(TOK, IN).T)  # [IN, TOK] fp32
    x8 = x_t.astype(f8)
    x_aug = np.empty((K_AUG, TOK), dtype=f8)
    x_aug[:IN] = x8
    resid = (x_t[:C_CORR] - x8[:C_CORR].astype(np.float32)) * RESID_SCALE
    x_aug[IN:] = resid.astype(f8)

    return [
        {
            "x_t": x_aug,
            "w_t": np.ascontiguousarray(w_aug[:, i * N_SHARD : (i + 1) * N_SHARD]),
        }
        for i in range(N_CORES)
    ]


def _postprocess(outs: list, bias: np.ndarray) -> np.ndarray:
    out = np.concatenate([np.asarray(o) for o in outs], axis=1)  # [TOK, OUT] f32
    out = out.reshape(B, S, OUT)
    if np.any(bias):
        out = out + bias.astype(np.float32)
    return out


def _ensure_ntff_hook_shim():
    """concourse's trace path imports antenv.axon_hooks, which is missing in
    this image. Provide the same ctypes-based hook (see trn_agent_boot) so a
    globally-set BASS_TRACE can't crash the run."""
    import sys

    try:
        import antenv.axon_hooks  # noqa: F401
        return
    except ImportError:
        pass

    import contextlib
    import ctypes
    import types

    def _make_hook():
        try:
            lib = ctypes.CDLL("/opt/axon/libaxon_pjrt.so")
        except OSError:
            return None
        if not hasattr(lib, "axon_start_nrt_profile"):
            return None
        lib.axon_start_nrt_profile.argtypes = [
            ctypes.POINTER(ctypes.c_int64), ctypes.c_size_t,
        ]
        lib.axon_start_nrt_profile.restype = ctypes.c_int64
        lib.axon_stop_nrt_profile.argtypes = [ctypes.c_char_p]
        lib.axon_stop_nrt_profile.restype = ctypes.c_int64

        @contextlib.contextmanager
        def _hook(output_dir, device_ids):
            import jax

            jax.devices()
            if device_ids:
                ids = (ctypes.c_int64 * len(device_ids))(*device_ids)
                rc = lib.axon_start_nrt_profile(ids, len(device_ids))
            else:
                rc = lib.axon_start_nrt_profile(None, 0)
            if rc != 0:
                raise RuntimeError(f"axon_start_nrt_profile rc={rc}")
            try:
                yield
            finally:
                lib.axon_stop_nrt_profile(str(output_dir).encode())

        return _hook

    hook = _make_hook()
    mod = types.ModuleType("antenv.axon_hooks")
    mod.get_axon_ntff_profile_hook = lambda: hook
    mod.set_axon_ntff_profile_hook = lambda h: None
    sys.modules["antenv.axon_hooks"] = mod
    try:
        import antenv

        antenv.axon_hooks = mod
    except ImportError:
        pass


def kernel(x: np.ndarray, weight: np.ndarray, bias: np.ndarray) -> np.ndarray:
    from concourse.bass_utils import run_bass_kernel_spmd

    x = np.asarray(x, dtype=np.float32)
    weight = np.asarray(weight, dtype=np.float32)
    bias = np.asarray(bias, dtype=np.float32)

    _ensure_ntff_hook_shim()
    in_maps = _prepare_in_maps(x, weight)
    nc = _get_nc()
    try:
        res = run_bass_kernel_spmd(nc, in_maps, core_ids=list(range(N_CORES)))
    except Exception:
        # transient NRT execute failures have been observed to clear on retry
        import time as _time

        _time.sleep(5)
        res = run_bass_kernel_spmd(nc, in_maps, core_ids=list(range(N_CORES)))
    return _postprocess([r["out"] for r in res.results], bias)


# revision 7
# speedup vs baseline: 1.2854x; 1.0520x over previous
"""BitLinear-1.58 forward on 8 trn2 NeuronCores.

out = x @ qw.T + bias, qw = clip(round(w / (eps + mean|w|)), -1, 1).

Strategy (fp8 DoubleRow, 2x PE rate vs bf16):
  - Quantize the weight on host (bit-identical to the reference), transpose
    to [in, out]; ternary values are exact in fp8 e4m3.
  - Quantize x to e4m3. Plain e4m3 x gives rel_err ~2.3e-2 (> the 2e-2
    gate), so augment the contraction: for the first C_CORR of the 2048
    K-indices, append residual rows 64*(x - e4m3(x)) paired with weight rows
    w/64 (both exact in e4m3). This cuts the error to ~1.65e-2 at
    (2048+C_CORR)/2048 of the fp8 cost.
  - Column-parallel across 8 cores: each core computes the full-token output
    for a 1024-wide slice of out_features with a Bass/Tile PE matmul
    (fp8 e4m3 inputs, DoubleRow perf mode, fp32 PSUM accumulation).
  - Concatenate the 8 output slices.
"""

import numpy as np
import ml_dtypes

B, S, IN, OUT = 4, 2048, 2048, 8192
N_CORES = 8
TOK = B * S
N_SHARD = OUT // N_CORES
SCALE_EPS = 1e-05

C_CORR = 768  # K-indices that get an exact-residual correction row
K_AUG = IN + C_CORR
RESID_SCALE = 64.0  # power of two: w/RESID_SCALE stays exact in e4m3

_CACHED_NC = None


def _build_nc():
    import concourse.mybir as mybir
    import concourse.tile as tile
    from concourse import bacc

    KO = K_AUG // 128  # 128-partition K slabs
    NS = KO // 2  # DoubleRow K slabs (256 logical K each)
    MT = TOK // 512  # token tiles
    TS = 4  # 128-token subtiles per token tile
    NH = N_SHARD // 512  # 512-wide output column halves

    nc = bacc.Bacc(None, target_bir_lowering=False)

    x_t = nc.dram_tensor("x_t", [K_AUG, TOK], mybir.dt.float8e4, kind="ExternalInput")
    w_t = nc.dram_tensor(
        "w_t", [K_AUG, N_SHARD], mybir.dt.float8e4, kind="ExternalInput"
    )
    out = nc.dram_tensor("out", [TOK, N_SHARD], mybir.dt.float32, kind="ExternalOutput")

    xv = x_t[:, :].rearrange("(po pi) t -> pi po t", pi=128)  # [128, KO, TOK]
    wv = w_t[:, :].rearrange("(po pi) n -> pi po n", pi=128)  # [128, KO, N_SHARD]
    ov = out[:, :].rearrange(
        "(mo ts p) n -> p mo ts n", ts=TS, p=128
    )  # [128, MT, TS, N_SHARD]

    DR = mybir.MatmulPerfMode.DoubleRow

    with tile.TileContext(nc) as tc:
        with (
            tc.tile_pool(name="warm", bufs=1) as warm_pool,
            tc.tile_pool(name="wpool", bufs=1) as wpool,
            tc.tile_pool(name="xpool", bufs=3) as xpool,
            tc.tile_pool(name="opool", bufs=3) as opool,
            tc.tile_pool(name="psum", bufs=2, space="PSUM") as psum,
        ):
            # The full weight shard stays SBUF-resident (KO KB per partition).
            # Interleave per-K-slab weight and first-x-tile DMAs so the first
            # matmul round is gated only on its first slab, and the rest of
            # the fill overlaps with compute.
            w_sb = wpool.tile([128, KO, N_SHARD], mybir.dt.float8e4)
            xt0 = xpool.tile([128, KO, 512], mybir.dt.float8e4, name="xt")
            for ks in range(NS):
                sl = slice(2 * ks, 2 * ks + 2)
                nc.sync.dma_start(w_sb[:, sl, :], wv[:, sl, :])
                nc.sync.dma_start(xt0[:, sl, :], xv[:, sl, 0:512])

            # PE warm-up: dummy matmuls with no data deps run while the first
            # input slabs are still DMA-ing in, so the HAM clock gate is
            # already released (2.4 GHz) when the real matmul stream starts.
            # The warm-up psum tile comes from the same rotating pool the
            # real accumulations use, keeping total PSUM usage at 8 banks.
            wl = warm_pool.tile([128, 512], mybir.dt.bfloat16)
            wp = psum.tile([128, 512], mybir.dt.float32, name="ps0")
            nc.vector.memset(wl[:], 0.0)
            n_warm = 8
            for i in range(n_warm):
                nc.tensor.matmul(
                    wp[:], wl[:, :128], wl[:], start=(i == 0), stop=(i == n_warm - 1)
                )

            for mt in range(MT):
                if mt == 0:
                    xt = xt0
                else:
                    xt = xpool.tile([128, KO, 512], mybir.dt.float8e4, name="xt")
                    nc.sync.dma_start(xt[:], xv[:, :, mt * 512 : (mt + 1) * 512])
                for nh in range(NH):
                    pss = [
                        psum.tile([128, 512], mybir.dt.float32, name=f"ps{t}")
                        for t in range(TS)
                    ]
                    for ks in range(NS):
                        for t in range(TS):
                            nc.tensor.matmul(
                                pss[t],
                                xt[:, 2 * ks : 2 * ks + 2, t * 128 : (t + 1) * 128],
                                w_sb[:, 2 * ks : 2 * ks + 2, nh * 512 : (nh + 1) * 512],
                                start=(ks == 0),
                                stop=(ks == NS - 1),
                                perf_mode=DR,
                            )
                    ot = opool.tile([128, TS, 512], mybir.dt.float32, name="ot")
                    for t in range(TS):
                        if t % 2 == 0:
                            nc.scalar.copy(out=ot[:, t, :], in_=pss[t][:])
                        else:
                            nc.vector.tensor_copy(out=ot[:, t, :], in_=pss[t][:])
                    nc.sync.dma_start(
                        ov[:, mt, :, nh * 512 : (nh + 1) * 512], ot[:]
                    )

    nc.compile()
    return nc


def _get_nc():
    global _CACHED_NC
    if _CACHED_NC is None:
        _CACHED_NC = _build_nc()
    return _CACHED_NC


def _quantize_weight(weight: np.ndarray) -> np.ndarray:
    """Ternarize exactly as the reference does (same jax ops, same backend)."""
    import jax.numpy as jnp

    w = jnp.asarray(weight)
    scale = SCALE_EPS + jnp.mean(jnp.abs(w))
    quant = jnp.clip(jnp.round(w / scale), -1.0, 1.0)
    return np.asarray(quant, dtype=np.float32)


def _prepare_in_maps(x: np.ndarray, weight: np.ndarray):
    f8 = ml_dtypes.float8_e4m3
    qw = _quantize_weight(weight)  # [OUT, IN] ternary fp32

    # weight side: [IN, OUT] ternary plus [C_CORR, OUT] scaled-down copy
    w_t = np.ascontiguousarray(qw.T)  # [IN, OUT] fp32
    w_aug = np.empty((K_AUG, OUT), dtype=f8)
    w_aug[:IN] = w_t.astype(f8)  # exact: -1/0/1
    w_aug[IN:] = (w_t[:C_CORR] / RESID_SCALE).astype(f8)  # exact: +-2^-6

    # x side: [IN, TOK] e4m3 plus [C_CORR, TOK] scaled residual
    x_t = np.ascontiguousarray(x.reshape(TOK, IN).T)  # [IN, TOK] fp32
    x8 = x_t.astype(f8)
    x_aug = np.empty((K_AUG, TOK), dtype=f8)
    x_aug[:IN] = x8
    resid = (x_t[:C_CORR] - x8[:C_CORR].astype(np.float32)) * RESID_SCALE
    x_aug[IN:] = resid.astype(f8)

    return [
        {
            "x_t": x_aug,
            "w_t": np.ascontiguousarray(w_aug[:, i * N_SHARD : (i + 1) * N_SHARD]),
        }
        for i in range(N_CORES)
    ]


def _postprocess(outs: list, bias: np.ndarray) -> np.ndarray:
    out = np.concatenate([np.asarray(o) for o in outs], axis=1)  # [TOK, OUT] f32
    out = out.reshape(B, S, OUT)
    if np.any(bias):
        out = out + bias.astype(np.float32)
    return out


def _ensure_ntff_hook_shim():
    """concourse's trace path imports antenv.axon_hooks, which is missing in
    this image. Provide the same ctypes-based hook (see trn_agent_boot) so a
    globally-set BASS_TRACE can't crash the run."""
    import sys

    try:
        import antenv.axon_hooks  # noqa: F401
        return
    except ImportError:
        pass

    import contextlib
    import ctypes
    import types

    def _make_hook():
        try:
            lib = ctypes.CDLL("/opt/axon/libaxon_pjrt.so")
        except OSError:
            return None
        if not hasattr(lib, "axon_start_nrt_profile"):
            return None
        lib.axon_start_nrt_profile.argtypes = [
            ctypes.POINTER(ctypes.c_int64), ctypes.c_size_t,
        ]
        lib.axon_start_nrt_profile.restype = ctypes.c_int64
        lib.axon_stop_nrt_profile.argtypes = [ctypes.c_char_p]
        lib.axon_stop_nrt_profile.restype = ctypes.c_int64

        @contextlib.contextmanager
        def _hook(output_dir, device_ids):
            import jax

            jax.devices()
            if device_ids:
                ids = (ctypes.c_int64 * len(device_ids))(*device_ids)
                rc = lib.axon_start_nrt_profile(ids, len(device_ids))
            else:
                rc = lib.axon_start_nrt_profile(None, 0)
            if rc != 0:
                raise RuntimeError(f"axon_start_nrt_profile rc={rc}")
            try:
                yield
            finally:
                lib.axon_stop_nrt_profile(str(output_dir).encode())

        return _hook

    hook = _make_hook()
    mod = types.ModuleType("antenv.axon_hooks")
    mod.get_axon_ntff_profile_hook = lambda: hook
    mod.set_axon_ntff_profile_hook = lambda h: None
    sys.modules["antenv.axon_hooks"] = mod
    try:
        import antenv

        antenv.axon_hooks = mod
    except ImportError:
        pass


def kernel(x: np.ndarray, weight: np.ndarray, bias: np.ndarray) -> np.ndarray:
    from concourse.bass_utils import run_bass_kernel_spmd

    x = np.asarray(x, dtype=np.float32)
    weight = np.asarray(weight, dtype=np.float32)
    bias = np.asarray(bias, dtype=np.float32)

    _ensure_ntff_hook_shim()
    in_maps = _prepare_in_maps(x, weight)
    nc = _get_nc()
    try:
        res = run_bass_kernel_spmd(nc, in_maps, core_ids=list(range(N_CORES)))
    except Exception:
        # transient NRT execute failures have been observed to clear on retry
        import time as _time

        _time.sleep(5)
        res = run_bass_kernel_spmd(nc, in_maps, core_ids=list(range(N_CORES)))
    return _postprocess([r["out"] for r in res.results], bias)
